# revision 1
# baseline (speedup 1.0000x reference)
"""Trainium2 Bass kernel for nn_CausalConv1d (depthwise causal conv, K=4).

Reference computation (T=8192, C=8448, K=4):
    padded = concat([state, inputs], axis=0)            # [T+K, C]
    out[t, c] = bias[c] + sum_j padded[t+j, c] * weight[c, j]
    updated_state = inputs[T-K:T]
    returns (out[:, :8192], out[:, 8192:8320], out[:, 8320:8448], updated_state)

Strategy:
  - Shard T across the 8 cores: core m computes out rows [m*1024, (m+1)*1024),
    consuming padded rows [m*1024, m*1024+1024+K-1) (halo of K-1=3 rows).
  - Host pre-transposes to channel-major [C, T_local] so SBUF tiles are
    [128 channels (partitions), time (free)] with fully contiguous DMA.
  - On device, the depthwise conv runs on the TensorEngine: for each group of
    128 channels, 4 matmuls with lhsT = diag(weight[:, j]) and rhs = the input
    tile read at free-offset j, accumulating the 4 taps in PSUM. ScalarE then
    adds the bias while copying PSUM -> SBUF.
  - Matmuls use the fp32r dtype (1 cycle/row vs 4 for fp32). fp32r = fp32
    rounded to 11 mantissa bits; inputs are pre-rounded on the host so device
    results are bit-deterministic (verified vs host emulation).
  - updated_state is a pure host-side slice of the inputs.
"""

import sys

sys.path.insert(0, "/opt/trn_rl_repo")

import numpy as np

import concourse.bass as bass
import concourse.mybir as mybir
from concourse import bacc
from concourse.tile import TileContext
from concourse.bass_utils import run_bass_kernel_spmd

# Problem shapes (hardcoded per the harness contract).
T, C, K = 8192, 8448, 4
N_CORES = 8
T_LOC = T // N_CORES  # 1024 output rows per core
TIN = T_LOC + K - 1  # 1027 input rows per core
P = 128  # SBUF partitions
G = C // P  # 66 channel groups
GB = 3  # channel groups per DMA batch
NI = G // GB  # 22 iterations
CHUNK = 512  # PSUM bank = 512 fp32
OUTPUT_DIMS = (8192, 128, 128)

_F32 = mybir.dt.float32
_F32R = mybir.dt.float32r


def _round_fp32r(a: np.ndarray) -> np.ndarray:
    """Round fp32 to 11 mantissa bits (round-to-nearest-even) — exactly what
    the PE's fp32r datapath does to its inputs (verified on hardware)."""
    ai = np.ascontiguousarray(a, dtype=np.float32).view(np.uint32)
    shift = 23 - 11
    bias = ((ai >> np.uint32(shift)) & np.uint32(1)) + np.uint32((1 << (shift - 1)) - 1)
    return (((ai + bias) >> np.uint32(shift)) << np.uint32(shift)).view(np.float32)


def _build():
    nc = bacc.Bacc("TRN2", target_bir_lowering=False, debug=False)

    x = nc.dram_tensor("x", [C, TIN], _F32R, kind="ExternalInput")
    w = nc.dram_tensor("w", [P, G * K], _F32, kind="ExternalInput")
    b = nc.dram_tensor("b", [P, G], _F32, kind="ExternalInput")
    ident = nc.dram_tensor("ident", [P, P], _F32, kind="ExternalInput")
    y = nc.dram_tensor("y", [C, T_LOC], _F32, kind="ExternalOutput")

    x_r = x[:].rearrange("(i g p) t -> i p g t", p=P, g=GB)  # [NI, P, GB, TIN]
    y_r = y[:].rearrange("(i g p) t -> i p g t", p=P, g=GB)  # [NI, P, GB, T_LOC]

    with TileContext(nc) as tc:
        with (
            tc.tile_pool(name="const", bufs=1) as cpool,
            tc.tile_pool(name="xin", bufs=3) as xpool,
            tc.tile_pool(name="yout", bufs=3) as ypool,
            tc.tile_pool(name="diag", bufs=16) as dpool,
            tc.tile_pool(name="psum", bufs=8, space=bass.MemorySpace.PSUM) as pspool,
        ):
            wt = cpool.tile([P, G * K], _F32)
            bt = cpool.tile([P, G], _F32)
            it = cpool.tile([P, P], _F32)
            nc.sync.dma_start(out=wt[:], in_=w[:])
            nc.sync.dma_start(out=bt[:], in_=b[:])
            nc.sync.dma_start(out=it[:], in_=ident[:])

            for i in range(NI):
                xt = xpool.tile([P, GB, TIN], _F32R)
                nc.sync.dma_start(out=xt[:], in_=x_r[i])
                yt = ypool.tile([P, GB, T_LOC], _F32)
                for g in range(GB):
                    gg = i * GB + g
                    diags = []
                    for j in range(K):
                        d = dpool.tile([P, P], _F32R, tag="diag")
                        nc.vector.tensor_scalar_mul(
                            d[:], it[:], wt[:, gg * K + j : gg * K + j + 1]
                        )
                        diags.append(d)
                    for c in range(T_LOC // CHUNK):
                        ps = pspool.tile([P, CHUNK], _F32)
                        for j in range(K):
                            nc.tensor.matmul(
                                ps[:],
                                diags[j][:],
                                xt[:, g, c * CHUNK + j : c * CHUNK + j + CHUNK],
                                start=(j == 0),
                                stop=(j == K - 1),
                            )
                        nc.scalar.add(
                            yt[:, g, c * CHUNK : (c + 1) * CHUNK],
                            ps[:],
                            bt[:, gg : gg + 1],
                        )
                nc.sync.dma_start(out=y_r[i], in_=yt[:])
    nc.compile()
    return nc


_NC_CACHE = None


def _get_nc():
    global _NC_CACHE
    if _NC_CACHE is None:
        _NC_CACHE = _build()
    return _NC_CACHE


def _prepare_in_maps(inputs, state, weight, bias):
    inputs = np.asarray(inputs, dtype=np.float32)
    state = np.asarray(state, dtype=np.float32)
    weight = np.asarray(weight, dtype=np.float32)
    bias = np.asarray(bias, dtype=np.float32)

    padded = np.concatenate([state, inputs], axis=0)  # [T+K, C]
    padded_r = _round_fp32r(padded)
    x_cm = np.ascontiguousarray(padded_r.T)  # [C, T+K]

    w_r = _round_fp32r(weight)  # [C, K]
    w_sb = np.ascontiguousarray(
        w_r.reshape(G, P, K).transpose(1, 0, 2).reshape(P, G * K)
    )
    b_sb = np.ascontiguousarray(bias.reshape(G, P).T)  # [P, G]
    ident = np.eye(P, dtype=np.float32)

    in_maps = []
    for m in range(N_CORES):
        xm = np.ascontiguousarray(x_cm[:, m * T_LOC : m * T_LOC + TIN])
        in_maps.append({"x": xm, "w": w_sb, "b": b_sb, "ident": ident})
    return in_maps


def kernel(inputs, state, weight, bias):
    nc = _get_nc()
    in_maps = _prepare_in_maps(inputs, state, weight, bias)
    res = run_bass_kernel_spmd(nc, in_maps, core_ids=list(range(N_CORES)))
    out_cm = np.concatenate([res.results[m]["y"] for m in range(N_CORES)], axis=1)
    out = out_cm.T  # [T, C]

    split_points = np.cumsum(OUTPUT_DIMS[:-1]).tolist()
    split_outputs = tuple(
        np.ascontiguousarray(s) for s in np.split(out, split_points, axis=-1)
    )
    updated_state = np.ascontiguousarray(inputs[T - K : T]).astype(np.float32)
    return (*split_outputs, updated_state)


# revision 2
# speedup vs baseline: 67.0623x; 67.0623x over previous
"""Trainium2 Bass kernel for nn_CausalConv1d (depthwise causal conv, K=4).

Reference computation (T=8192, C=8448, K=4):
    padded = concat([state, inputs], axis=0)            # [T+K, C]
    out[t, c] = bias[c] + sum_j padded[t+j, c] * weight[c, j]
    updated_state = inputs[T-K:T]
    returns (out[:, :8192], out[:, 8192:8320], out[:, 8320:8448], updated_state)

Strategy:
  - Shard T across the 8 cores: core m computes out rows [m*1024, (m+1)*1024),
    consuming padded rows [m*1024, m*1024+1024+K-1) (halo of K-1=3 rows).
  - Host pre-transposes to channel-major [C, T_local] so SBUF tiles are
    [128 channels (partitions), time (free)] with fully contiguous DMA.
  - On device, the depthwise conv runs on the TensorEngine: for each group of
    128 channels, 4 matmuls with lhsT = diag(weight[:, j]) and rhs = the input
    tile read at free-offset j, accumulating the 4 taps in PSUM. ScalarE then
    adds the bias while copying PSUM -> SBUF.
  - Matmuls use the fp32r dtype (1 cycle/row vs 4 for fp32). fp32r = fp32
    rounded to 11 mantissa bits; inputs are pre-rounded on the host so device
    results are bit-deterministic (verified vs host emulation).
  - updated_state is a pure host-side slice of the inputs.
"""

import sys

sys.path.insert(0, "/opt/trn_rl_repo")

import numpy as np

import concourse.bass as bass
import concourse.mybir as mybir
from concourse import bacc
from concourse.tile import TileContext
from concourse.bass_utils import run_bass_kernel_spmd

# Problem shapes (hardcoded per the harness contract).
T, C, K = 8192, 8448, 4
N_CORES = 8
T_LOC = T // N_CORES  # 1024 output rows per core
TIN = T_LOC + K - 1  # 1027 input rows per core
P = 128  # SBUF partitions
G = C // P  # 66 channel groups
GB = 3  # channel groups per DMA batch
NI = G // GB  # 22 iterations
CHUNK = 512  # PSUM bank = 512 fp32
OUTPUT_DIMS = (8192, 128, 128)

_F32 = mybir.dt.float32
_F32R = mybir.dt.float32r


def _round_fp32r(a: np.ndarray) -> np.ndarray:
    """Round fp32 to 11 mantissa bits (round-to-nearest-even) — exactly what
    the PE's fp32r datapath does to its inputs (verified on hardware)."""
    ai = np.ascontiguousarray(a, dtype=np.float32).view(np.uint32)
    shift = 23 - 11
    bias = ((ai >> np.uint32(shift)) & np.uint32(1)) + np.uint32((1 << (shift - 1)) - 1)
    return (((ai + bias) >> np.uint32(shift)) << np.uint32(shift)).view(np.float32)


def _build(loop_reps: int | None = None):
    """Build the per-core Bass program. loop_reps (timing only): wrap the whole
    body in a device-side For_i repeat so per-exec dispatch overhead can be
    subtracted out when benchmarking."""
    nc = bacc.Bacc("TRN2", target_bir_lowering=False, debug=False)

    x = nc.dram_tensor("x", [C, TIN], _F32R, kind="ExternalInput")
    w = nc.dram_tensor("w", [P, G * K], _F32, kind="ExternalInput")
    b = nc.dram_tensor("b", [P, G], _F32, kind="ExternalInput")
    ident = nc.dram_tensor("ident", [P, P], _F32, kind="ExternalInput")
    y = nc.dram_tensor("y", [C, T_LOC], _F32, kind="ExternalOutput")

    x_r = x[:].rearrange("(i g p) t -> i p g t", p=P, g=GB)  # [NI, P, GB, TIN]
    y_r = y[:].rearrange("(i g p) t -> i p g t", p=P, g=GB)  # [NI, P, GB, T_LOC]

    from contextlib import ExitStack

    with TileContext(nc) as tc:
        with (
            tc.tile_pool(name="const", bufs=1) as cpool,
            tc.tile_pool(name="xin", bufs=3) as xpool,
            tc.tile_pool(name="yout", bufs=3) as ypool,
            tc.tile_pool(name="diag", bufs=16) as dpool,
            tc.tile_pool(name="psum", bufs=8, space=bass.MemorySpace.PSUM) as pspool,
        ):
            wt = cpool.tile([P, G * K], _F32)
            bt = cpool.tile([P, G], _F32)
            it = cpool.tile([P, P], _F32)
            nc.sync.dma_start(out=wt[:], in_=w[:])
            nc.sync.dma_start(out=bt[:], in_=b[:])
            nc.sync.dma_start(out=it[:], in_=ident[:])

            with ExitStack() as loop_ctx:
                if loop_reps is not None:
                    loop_ctx.enter_context(tc.For_i(0, loop_reps, 1))
                for i in range(NI):
                    xt = xpool.tile([P, GB, TIN], _F32R)
                    nc.sync.dma_start(out=xt[:], in_=x_r[i])
                    yt = ypool.tile([P, GB, T_LOC], _F32)
                    for g in range(GB):
                        gg = i * GB + g
                        diags = []
                        for j in range(K):
                            d = dpool.tile([P, P], _F32R, tag="diag")
                            nc.vector.tensor_scalar_mul(
                                d[:], it[:], wt[:, gg * K + j : gg * K + j + 1]
                            )
                            diags.append(d)
                        for c in range(T_LOC // CHUNK):
                            ps = pspool.tile([P, CHUNK], _F32)
                            for j in range(K):
                                nc.tensor.matmul(
                                    ps[:],
                                    diags[j][:],
                                    xt[:, g, c * CHUNK + j : c * CHUNK + j + CHUNK],
                                    start=(j == 0),
                                    stop=(j == K - 1),
                                )
                            nc.scalar.add(
                                yt[:, g, c * CHUNK : (c + 1) * CHUNK],
                                ps[:],
                                bt[:, gg : gg + 1],
                            )
                    nc.sync.dma_start(out=y_r[i], in_=yt[:])
    nc.compile()
    return nc


_NC_CACHE = None


def _get_nc():
    global _NC_CACHE
    if _NC_CACHE is None:
        _NC_CACHE = _build()
    return _NC_CACHE


def _prepare_in_maps(inputs, state, weight, bias):
    inputs = np.asarray(inputs, dtype=np.float32)
    state = np.asarray(state, dtype=np.float32)
    weight = np.asarray(weight, dtype=np.float32)
    bias = np.asarray(bias, dtype=np.float32)

    padded = np.concatenate([state, inputs], axis=0)  # [T+K, C]
    padded_r = _round_fp32r(padded)
    x_cm = np.ascontiguousarray(padded_r.T)  # [C, T+K]

    w_r = _round_fp32r(weight)  # [C, K]
    w_sb = np.ascontiguousarray(
        w_r.reshape(G, P, K).transpose(1, 0, 2).reshape(P, G * K)
    )
    b_sb = np.ascontiguousarray(bias.reshape(G, P).T)  # [P, G]
    ident = np.eye(P, dtype=np.float32)

    in_maps = []
    for m in range(N_CORES):
        xm = np.ascontiguousarray(x_cm[:, m * T_LOC : m * T_LOC + TIN])
        in_maps.append({"x": xm, "w": w_sb, "b": b_sb, "ident": ident})
    return in_maps


def kernel(inputs, state, weight, bias):
    nc = _get_nc()
    in_maps = _prepare_in_maps(inputs, state, weight, bias)
    res = run_bass_kernel_spmd(nc, in_maps, core_ids=list(range(N_CORES)))
    out_cm = np.concatenate([res.results[m]["y"] for m in range(N_CORES)], axis=1)
    out = out_cm.T  # [T, C]

    split_points = np.cumsum(OUTPUT_DIMS[:-1]).tolist()
    split_outputs = tuple(
        np.ascontiguousarray(s) for s in np.split(out, split_points, axis=-1)
    )
    updated_state = np.ascontiguousarray(inputs[T - K : T]).astype(np.float32)
    return (*split_outputs, updated_state)


# revision 3
# speedup vs baseline: 161.8853x; 2.4140x over previous
"""Trainium2 Bass kernel for nn_CausalConv1d (depthwise causal conv, K=4).

Reference computation (T=8192, C=8448, K=4):
    padded = concat([state, inputs], axis=0)            # [T+K, C]
    out[t, c] = bias[c] + sum_j padded[t+j, c] * weight[c, j]
    updated_state = inputs[T-K:T]
    returns (out[:, :8192], out[:, 8192:8320], out[:, 8320:8448], updated_state)

Strategy:
  - Shard T across the 8 cores: core m computes out rows [m*1024, (m+1)*1024),
    consuming padded rows [m*1024, m*1024+1024+K-1) (halo of K-1=3 rows).
  - Host pre-permutes each core's input to the exact SBUF layout
    x_flat[p, g*1027 + t] = padded[m*1024 + t, g*128 + p], so every DMA is a
    plain 2D transfer with long contiguous per-partition runs.
  - On device, the depthwise conv runs on the TensorEngine: for each group of
    128 channels, 4 matmuls with lhsT = diag(weight[:, j]) and rhs = the input
    tile read at free-offset j, accumulating the 4 taps in PSUM. ScalarE then
    adds the bias while copying PSUM -> SBUF.
  - Matmuls use the fp32r dtype (1 cycle/row vs 4 for fp32). fp32r = fp32
    rounded to 11 mantissa bits; inputs are pre-rounded on the host so device
    results are bit-deterministic (verified vs host emulation).
  - Input DMAs issue on the SP HWDGE ring (nc.sync), output DMAs on the ACT
    ring (nc.scalar) so the two streams don't serialize on one FIFO.
  - updated_state is a pure host-side slice of the inputs.
"""

import sys

sys.path.insert(0, "/opt/trn_rl_repo")

from contextlib import ExitStack

import numpy as np

import concourse.bass as bass
import concourse.mybir as mybir
from concourse import bacc
from concourse.tile import TileContext
from concourse.bass_utils import run_bass_kernel_spmd

# Problem shapes (hardcoded per the harness contract).
T, C, K = 8192, 8448, 4
N_CORES = 8
T_LOC = T // N_CORES  # 1024 output rows per core
TIN = T_LOC + K - 1  # 1027 input rows per core
P = 128  # SBUF partitions
G = C // P  # 66 channel groups
GB = 6  # channel groups per DMA batch
NI = G // GB  # 11 iterations
CHUNK = 512  # PSUM bank = 512 fp32
OUTPUT_DIMS = (8192, 128, 128)

_F32 = mybir.dt.float32
_F32R = mybir.dt.float32r


def _round_fp32r(a: np.ndarray) -> np.ndarray:
    """Round fp32 to 11 mantissa bits (round-to-nearest-even) — exactly what
    the PE's fp32r datapath does to its inputs (verified on hardware)."""
    ai = np.ascontiguousarray(a, dtype=np.float32).view(np.uint32)
    shift = 23 - 11
    bias = ((ai >> np.uint32(shift)) & np.uint32(1)) + np.uint32((1 << (shift - 1)) - 1)
    return (((ai + bias) >> np.uint32(shift)) << np.uint32(shift)).view(np.float32)


def _build(loop_reps: int | None = None, mode: str = "full"):
    """Build the per-core Bass program.

    loop_reps (timing only): wrap the body in a device-side For_i repeat so
    per-exec dispatch overhead can be subtracted out when benchmarking.
    mode: 'full' | 'dma' (skip compute, out-DMA echoes input tile) |
          'compute' (load one tile, run all compute on it, one out-DMA).
    """
    nc = bacc.Bacc("TRN2", target_bir_lowering=False, debug=False)

    x = nc.dram_tensor("x", [P, G * TIN], _F32R, kind="ExternalInput")
    w = nc.dram_tensor("w", [P, G * K], _F32, kind="ExternalInput")
    b = nc.dram_tensor("b", [P, G], _F32, kind="ExternalInput")
    ident = nc.dram_tensor("ident", [P, P], _F32, kind="ExternalInput")
    y = nc.dram_tensor("y", [P, G * T_LOC], _F32, kind="ExternalOutput")

    with TileContext(nc) as tc:
        with (
            tc.tile_pool(name="const", bufs=1) as cpool,
            tc.tile_pool(name="xin", bufs=3) as xpool,
            tc.tile_pool(name="yout", bufs=2) as ypool,
            tc.tile_pool(name="diag", bufs=16) as dpool,
            tc.tile_pool(name="psum", bufs=8, space=bass.MemorySpace.PSUM) as pspool,
        ):
            wt = cpool.tile([P, G * K], _F32)
            bt = cpool.tile([P, G], _F32)
            it = cpool.tile([P, P], _F32)
            nc.sync.dma_start(out=wt[:], in_=w[:])
            nc.sync.dma_start(out=bt[:], in_=b[:])
            nc.sync.dma_start(out=it[:], in_=ident[:])

            with ExitStack() as loop_ctx:
                if loop_reps is not None:
                    loop_ctx.enter_context(tc.For_i(0, loop_reps, 1))
                for i in range(NI):
                    if mode == "compute" and i > 0:
                        xt = xt  # noqa: PLW0127 — reuse first tile
                    else:
                        xt = xpool.tile([P, GB * TIN], _F32R)
                        nc.sync.dma_start(
                            out=xt[:], in_=x[:, i * GB * TIN : (i + 1) * GB * TIN]
                        )
                    if mode == "dma":
                        nc.scalar.dma_start(
                            out=y[:, i * GB * T_LOC : (i + 1) * GB * T_LOC],
                            in_=xt[:, : GB * T_LOC].bitcast(_F32),
                        )
                        continue
                    yt = ypool.tile([P, GB * T_LOC], _F32)
                    for g in range(GB):
                        gg = i * GB + g
                        diags = []
                        for j in range(K):
                            d = dpool.tile([P, P], _F32R, tag="diag")
                            nc.vector.tensor_scalar_mul(
                                d[:], it[:], wt[:, gg * K + j : gg * K + j + 1]
                            )
                            diags.append(d)
                        for c in range(T_LOC // CHUNK):
                            ps = pspool.tile([P, CHUNK], _F32)
                            base = g * TIN + c * CHUNK
                            for j in range(K):
                                nc.tensor.matmul(
                                    ps[:],
                                    diags[j][:],
                                    xt[:, base + j : base + j + CHUNK],
                                    start=(j == 0),
                                    stop=(j == K - 1),
                                )
                            nc.scalar.add(
                                yt[:, g * T_LOC + c * CHUNK : g * T_LOC + (c + 1) * CHUNK],
                                ps[:],
                                bt[:, gg : gg + 1],
                            )
                    if mode == "compute" and i < NI - 1:
                        continue
                    nc.scalar.dma_start(
                        out=y[:, i * GB * T_LOC : (i + 1) * GB * T_LOC], in_=yt[:]
                    )
    nc.compile()
    return nc


_NC_CACHE = None


def _get_nc():
    global _NC_CACHE
    if _NC_CACHE is None:
        _NC_CACHE = _build()
    return _NC_CACHE


def _prepare_in_maps(inputs, state, weight, bias):
    inputs = np.asarray(inputs, dtype=np.float32)
    state = np.asarray(state, dtype=np.float32)
    weight = np.asarray(weight, dtype=np.float32)
    bias = np.asarray(bias, dtype=np.float32)

    padded = np.concatenate([state, inputs], axis=0)  # [T+K, C]
    padded_r = _round_fp32r(padded)

    w_r = _round_fp32r(weight)  # [C, K]
    w_sb = np.ascontiguousarray(
        w_r.reshape(G, P, K).transpose(1, 0, 2).reshape(P, G * K)
    )
    b_sb = np.ascontiguousarray(bias.reshape(G, P).T)  # [P, G]
    ident = np.eye(P, dtype=np.float32)

    in_maps = []
    for m in range(N_CORES):
        seg = padded_r[m * T_LOC : m * T_LOC + TIN]  # [TIN, C]
        # x_flat[p, g*TIN + t] = seg[t, g*128 + p]
        xm = np.ascontiguousarray(
            seg.reshape(TIN, G, P).transpose(2, 1, 0).reshape(P, G * TIN)
        )
        in_maps.append({"x": xm, "w": w_sb, "b": b_sb, "ident": ident})
    return in_maps


def _assemble_out(results):
    """results[m]['y'] is [P, G*T_LOC] with y[p, g*T_LOC + t] = out[m*T_LOC+t, g*128+p]."""
    out = np.empty((T, C), dtype=np.float32)
    for m in range(N_CORES):
        yf = results[m]["y"].reshape(P, G, T_LOC)
        out[m * T_LOC : (m + 1) * T_LOC] = yf.transpose(2, 1, 0).reshape(T_LOC, C)
    return out


def kernel(inputs, state, weight, bias):
    nc = _get_nc()
    in_maps = _prepare_in_maps(inputs, state, weight, bias)
    res = run_bass_kernel_spmd(nc, in_maps, core_ids=list(range(N_CORES)))
    out = _assemble_out(res.results)

    split_points = np.cumsum(OUTPUT_DIMS[:-1]).tolist()
    split_outputs = tuple(
        np.ascontiguousarray(s) for s in np.split(out, split_points, axis=-1)
    )
    updated_state = np.ascontiguousarray(np.asarray(inputs, dtype=np.float32)[T - K : T])
    return (*split_outputs, updated_state)


# revision 6
# speedup vs baseline: 244.3159x; 1.5092x over previous
"""Trainium2 Bass kernel for nn_CausalConv1d (depthwise causal conv, K=4).

Reference computation (T=8192, C=8448, K=4):
    padded = concat([state, inputs], axis=0)            # [T+K, C]
    out[t, c] = bias[c] + sum_j padded[t+j, c] * weight[c, j]
    updated_state = inputs[T-K:T]
    returns (out[:, :8192], out[:, 8192:8320], out[:, 8320:8448], updated_state)

Strategy:
  - Shard T across the 8 cores: core m computes out rows [m*1024, (m+1)*1024),
    consuming padded rows [m*1024, m*1024+1024+K-1) (halo of K-1=3 rows).
  - Host pre-permutes each core's input to the exact SBUF layout
    x_flat[p, g*1027 + t] = padded[m*1024 + t, g*128 + p], so every DMA is a
    plain 2D transfer with long contiguous per-partition runs.
  - On device, the depthwise conv runs on the TensorEngine: for each group of
    128 channels, 4 matmuls with lhsT = diag(weight[:, j]) and rhs = the input
    tile read at free-offset j, accumulating the 4 taps in PSUM. ScalarE then
    adds the bias while copying PSUM -> SBUF.
  - Matmuls use the fp32r dtype (1 cycle/row vs 4 for fp32). fp32r = fp32
    rounded to 11 mantissa bits; inputs are pre-rounded on the host so device
    results are bit-deterministic (verified vs host emulation).
  - Input DMAs issue on the SP HWDGE ring (nc.sync), output DMAs on the ACT
    ring (nc.scalar) so the two streams don't serialize on one FIFO.
  - updated_state is a pure host-side slice of the inputs.
"""

import sys

sys.path.insert(0, "/opt/trn_rl_repo")

from contextlib import ExitStack

import numpy as np

import concourse.bass as bass
import concourse.mybir as mybir
from concourse import bacc
from concourse.tile import TileContext
from concourse.bass_utils import run_bass_kernel_spmd

# Problem shapes (hardcoded per the harness contract).
T, C, K = 8192, 8448, 4
N_CORES = 8
T_LOC = T // N_CORES  # 1024 output rows per core
TIN = T_LOC + K - 1  # 1027 input rows per core
P = 128  # SBUF partitions
G = C // P  # 66 channel groups
GB = 6  # channel groups per DMA batch
NI = G // GB  # 11 iterations
CHUNK = 512  # PSUM bank = 512 fp32
OUTPUT_DIMS = (8192, 128, 128)

_F32 = mybir.dt.float32
_F32R = mybir.dt.float32r


def _round_fp32r(a: np.ndarray) -> np.ndarray:
    """Round fp32 to 11 mantissa bits (round-to-nearest-even) — exactly what
    the PE's fp32r datapath does to its inputs (verified on hardware)."""
    ai = np.ascontiguousarray(a, dtype=np.float32).view(np.uint32)
    shift = 23 - 11
    bias = ((ai >> np.uint32(shift)) & np.uint32(1)) + np.uint32((1 << (shift - 1)) - 1)
    return (((ai + bias) >> np.uint32(shift)) << np.uint32(shift)).view(np.float32)


def _build(loop_reps: int | None = None, mode: str = "full", dtype: str = "fp16"):
    """Build the per-core Bass program.

    loop_reps (timing only): wrap the body in a device-side For_i repeat so
    per-exec dispatch overhead can be subtracted out when benchmarking.
    mode: 'full' | 'dma' (skip compute, out-DMA echoes input tile) |
          'compute' (load one tile, run all compute on it, one out-DMA).
    dtype: 'fp16' (x/y/diag in fp16 — half the HBM traffic, 10-bit mantissa)
           or 'f32r' (x/diag fp32r, y fp32 — 11-bit mantissa products).
    """
    nc = bacc.Bacc("TRN2", target_bir_lowering=False, debug=False)

    if dtype == "fp16":
        xdt = ydt = mybir.dt.float16
    else:
        xdt, ydt = _F32R, _F32

    x = nc.dram_tensor("x", [P, G * TIN], xdt, kind="ExternalInput")
    w = nc.dram_tensor("w", [P, G * K], _F32, kind="ExternalInput")
    b = nc.dram_tensor("b", [P, G], _F32, kind="ExternalInput")
    ident = nc.dram_tensor("ident", [P, P], _F32, kind="ExternalInput")
    y = nc.dram_tensor("y", [P, G * T_LOC], ydt, kind="ExternalOutput")

    with TileContext(nc) as tc:
        with (
            tc.tile_pool(name="const", bufs=1) as cpool,
            tc.tile_pool(name="xin", bufs=3) as xpool,
            tc.tile_pool(name="yout", bufs=2) as ypool,
            tc.tile_pool(name="diag", bufs=16) as dpool,
            tc.tile_pool(name="psum", bufs=8, space=bass.MemorySpace.PSUM) as pspool,
        ):
            wt = cpool.tile([P, G * K], _F32)
            bt = cpool.tile([P, G], _F32)
            it = cpool.tile([P, P], _F32)
            nc.sync.dma_start(out=wt[:], in_=w[:])
            nc.sync.dma_start(out=bt[:], in_=b[:])
            nc.sync.dma_start(out=it[:], in_=ident[:])

            with ExitStack() as loop_ctx:
                if loop_reps is not None:
                    loop_ctx.enter_context(tc.For_i(0, loop_reps, 1))
                for i in range(NI):
                    if mode == "compute" and i > 0:
                        xt = xt  # noqa: PLW0127 — reuse first tile
                    else:
                        xt = xpool.tile([P, GB * TIN], xdt)
                        nc.sync.dma_start(
                            out=xt[:], in_=x[:, i * GB * TIN : (i + 1) * GB * TIN]
                        )
                    if mode == "dma":
                        nc.scalar.dma_start(
                            out=y[:, i * GB * T_LOC : (i + 1) * GB * T_LOC],
                            in_=xt[:, : GB * T_LOC].bitcast(ydt),
                        )
                        continue
                    yt = ypool.tile([P, GB * T_LOC], ydt)
                    for g in range(GB):
                        gg = i * GB + g
                        diags = []
                        for j in range(K):
                            d = dpool.tile([P, P], xdt, tag="diag")
                            nc.vector.tensor_scalar_mul(
                                d[:], it[:], wt[:, gg * K + j : gg * K + j + 1]
                            )
                            diags.append(d)
                        for c in range(T_LOC // CHUNK):
                            ps = pspool.tile([P, CHUNK], _F32)
                            base = g * TIN + c * CHUNK
                            for j in range(K):
                                nc.tensor.matmul(
                                    ps[:],
                                    diags[j][:],
                                    xt[:, base + j : base + j + CHUNK],
                                    start=(j == 0),
                                    stop=(j == K - 1),
                                )
                            nc.scalar.add(
                                yt[:, g * T_LOC + c * CHUNK : g * T_LOC + (c + 1) * CHUNK],
                                ps[:],
                                bt[:, gg : gg + 1],
                            )
                    if mode == "compute" and i < NI - 1:
                        continue
                    nc.scalar.dma_start(
                        out=y[:, i * GB * T_LOC : (i + 1) * GB * T_LOC], in_=yt[:]
                    )
    nc.compile()
    return nc


DTYPE = "fp16"  # 'fp16' or 'f32r'

_NC_CACHE = None


def _get_nc():
    global _NC_CACHE
    if _NC_CACHE is None:
        _NC_CACHE = _build(dtype=DTYPE)
    return _NC_CACHE


def _prepare_in_maps(inputs, state, weight, bias):
    inputs = np.asarray(inputs, dtype=np.float32)
    state = np.asarray(state, dtype=np.float32)
    weight = np.asarray(weight, dtype=np.float32)
    bias = np.asarray(bias, dtype=np.float32)

    padded = np.concatenate([state, inputs], axis=0)  # [T+K, C]
    if DTYPE == "fp16":
        padded_r = padded.astype(np.float16)
        w_r = weight  # diag build rounds to fp16 on the DVE write
    else:
        padded_r = _round_fp32r(padded)
        w_r = _round_fp32r(weight)

    w_sb = np.ascontiguousarray(
        w_r.reshape(G, P, K).transpose(1, 0, 2).reshape(P, G * K)
    )
    b_sb = np.ascontiguousarray(bias.reshape(G, P).T)  # [P, G]
    ident = np.eye(P, dtype=np.float32)

    in_maps = []
    for m in range(N_CORES):
        seg = padded_r[m * T_LOC : m * T_LOC + TIN]  # [TIN, C]
        # x_flat[p, g*TIN + t] = seg[t, g*128 + p]
        xm = np.ascontiguousarray(
            seg.reshape(TIN, G, P).transpose(2, 1, 0).reshape(P, G * TIN)
        )
        in_maps.append({"x": xm, "w": w_sb, "b": b_sb, "ident": ident})
    return in_maps


def _assemble_out(results):
    """results[m]['y'] is [P, G*T_LOC] with y[p, g*T_LOC + t] = out[m*T_LOC+t, g*128+p]."""
    out = np.empty((T, C), dtype=np.float32)
    for m in range(N_CORES):
        yf = results[m]["y"].astype(np.float32).reshape(P, G, T_LOC)
        out[m * T_LOC : (m + 1) * T_LOC] = yf.transpose(2, 1, 0).reshape(T_LOC, C)
    return out


def kernel(inputs, state, weight, bias):
    nc = _get_nc()
    in_maps = _prepare_in_maps(inputs, state, weight, bias)
    res = run_bass_kernel_spmd(nc, in_maps, core_ids=list(range(N_CORES)))
    out = _assemble_out(res.results)

    split_points = np.cumsum(OUTPUT_DIMS[:-1]).tolist()
    split_outputs = tuple(
        np.ascontiguousarray(s) for s in np.split(out, split_points, axis=-1)
    )
    updated_state = np.ascontiguousarray(np.asarray(inputs, dtype=np.float32)[T - K : T])
    return (*split_outputs, updated_state)


# revision 9
# speedup vs baseline: 416.6047x; 1.7052x over previous
"""Trainium2 Bass kernel for nn_CausalConv1d (depthwise causal conv, K=4).

Reference computation (T=8192, C=8448, K=4):
    padded = concat([state, inputs], axis=0)            # [T+K, C]
    out[t, c] = bias[c] + sum_j padded[t+j, c] * weight[c, j]
    updated_state = inputs[T-K:T]
    returns (out[:, :8192], out[:, 8192:8320], out[:, 8320:8448], updated_state)

Strategy:
  - Shard T across the 8 cores: core m computes out rows [m*1024, (m+1)*1024),
    consuming padded rows [m*1024, m*1024+1024+K-1) (halo of K-1=3 rows).
  - Host pre-permutes each core's input to the exact SBUF layout
    x_flat[p, g*1027 + t] = padded[m*1024 + t, g*128 + p], so every DMA is a
    plain 2D transfer with long contiguous per-partition runs.
  - On device, the depthwise conv runs on the TensorEngine: for each group of
    128 channels, 4 matmuls with lhsT = diag(weight[:, j]) and rhs = the input
    tile read at free-offset j, accumulating the 4 taps in PSUM. ScalarE then
    adds the bias while copying PSUM -> SBUF.
  - Matmuls use the fp32r dtype (1 cycle/row vs 4 for fp32). fp32r = fp32
    rounded to 11 mantissa bits; inputs are pre-rounded on the host so device
    results are bit-deterministic (verified vs host emulation).
  - Input DMAs issue on the SP HWDGE ring (nc.sync), output DMAs on the ACT
    ring (nc.scalar) so the two streams don't serialize on one FIFO.
  - updated_state is a pure host-side slice of the inputs.
"""

import sys

sys.path.insert(0, "/opt/trn_rl_repo")

from contextlib import ExitStack

import numpy as np

import concourse.bass as bass
import concourse.mybir as mybir
from concourse import bacc
from concourse.tile import TileContext
from concourse.bass_utils import run_bass_kernel_spmd

# Problem shapes (hardcoded per the harness contract).
T, C, K = 8192, 8448, 4
N_CORES = 8
T_LOC = T // N_CORES  # 1024 output rows per core
TIN = T_LOC + K - 1  # 1027 input rows per core
P = 128  # SBUF partitions
G = C // P  # 66 channel groups
GB = 6  # channel groups per DMA batch
NI = G // GB  # 11 iterations
CHUNK = 512  # PSUM bank = 512 fp32
OUTPUT_DIMS = (8192, 128, 128)

_F32 = mybir.dt.float32
_F32R = mybir.dt.float32r


def _round_fp32r(a: np.ndarray) -> np.ndarray:
    """Round fp32 to 11 mantissa bits (round-to-nearest-even) — exactly what
    the PE's fp32r datapath does to its inputs (verified on hardware)."""
    ai = np.ascontiguousarray(a, dtype=np.float32).view(np.uint32)
    shift = 23 - 11
    bias = ((ai >> np.uint32(shift)) & np.uint32(1)) + np.uint32((1 << (shift - 1)) - 1)
    return (((ai + bias) >> np.uint32(shift)) << np.uint32(shift)).view(np.float32)


def _build(loop_reps: int | None = None, mode: str = "full", dtype: str = "fp16"):
    """Build the per-core Bass program.

    loop_reps (timing only): wrap the body in a device-side For_i repeat so
    per-exec dispatch overhead can be subtracted out when benchmarking.
    mode: 'full' | 'dma' (skip compute, out-DMA echoes input tile) |
          'compute' (load one tile, run all compute on it, one out-DMA).
    dtype: 'fp16' (x/y/diag in fp16 — half the HBM traffic, 10-bit mantissa)
           or 'f32r' (x/diag fp32r, y fp32 — 11-bit mantissa products).
    """
    nc = bacc.Bacc("TRN2", target_bir_lowering=False, debug=False)

    if dtype == "fp16":
        xdt = ydt = mybir.dt.float16
    else:
        xdt, ydt = _F32R, _F32

    x = nc.dram_tensor("x", [P, G * TIN], xdt, kind="ExternalInput")
    w = nc.dram_tensor("w", [P, G * K], _F32, kind="ExternalInput")
    b = nc.dram_tensor("b", [P, G], _F32, kind="ExternalInput")
    ident = nc.dram_tensor("ident", [P, P], _F32, kind="ExternalInput")
    y = nc.dram_tensor("y", [P, G * T_LOC], ydt, kind="ExternalOutput")

    with TileContext(nc) as tc:
        with (
            tc.tile_pool(name="const", bufs=1) as cpool,
            tc.tile_pool(name="xin", bufs=3) as xpool,
            tc.tile_pool(name="yout", bufs=2) as ypool,
            tc.tile_pool(name="diag", bufs=16) as dpool,
            tc.tile_pool(name="psum", bufs=8, space=bass.MemorySpace.PSUM) as pspool,
        ):
            wt = cpool.tile([P, G * K], _F32)
            bt = cpool.tile([P, G], _F32)
            it = cpool.tile([P, P], _F32)
            nc.sync.dma_start(out=wt[:], in_=w[:])
            nc.sync.dma_start(out=bt[:], in_=b[:])
            nc.sync.dma_start(out=it[:], in_=ident[:])

            with ExitStack() as loop_ctx:
                if loop_reps is not None:
                    loop_ctx.enter_context(tc.For_i(0, loop_reps, 1))
                for i in range(NI):
                    if mode == "compute" and i > 0:
                        xt = xt  # noqa: PLW0127 — reuse first tile
                    else:
                        xt = xpool.tile([P, GB * TIN], xdt)
                        nc.sync.dma_start(
                            out=xt[:], in_=x[:, i * GB * TIN : (i + 1) * GB * TIN]
                        )
                    if mode == "dma":
                        nc.scalar.dma_start(
                            out=y[:, i * GB * T_LOC : (i + 1) * GB * T_LOC],
                            in_=xt[:, : GB * T_LOC].bitcast(ydt),
                        )
                        continue
                    yt = ypool.tile([P, GB * T_LOC], ydt)
                    for g in range(GB):
                        gg = i * GB + g
                        diags = []
                        for j in range(K):
                            d = dpool.tile([P, P], xdt, tag="diag")
                            nc.vector.tensor_scalar_mul(
                                d[:], it[:], wt[:, gg * K + j : gg * K + j + 1]
                            )
                            diags.append(d)
                        # j-outer: consecutive matmuls share the same stationary
                        # weights, halving LDWEIGHTS traffic on the PE.
                        pss = [
                            pspool.tile([P, CHUNK], _F32, name="ps", tag="ps")
                            for _ in range(T_LOC // CHUNK)
                        ]
                        for j in range(K):
                            for c in range(T_LOC // CHUNK):
                                base = g * TIN + c * CHUNK
                                nc.tensor.matmul(
                                    pss[c][:],
                                    diags[j][:],
                                    xt[:, base + j : base + j + CHUNK],
                                    start=(j == 0),
                                    stop=(j == K - 1),
                                    skip_group_check=True,
                                )
                        for c in range(T_LOC // CHUNK):
                            nc.scalar.add(
                                yt[:, g * T_LOC + c * CHUNK : g * T_LOC + (c + 1) * CHUNK],
                                pss[c][:],
                                bt[:, gg : gg + 1],
                            )
                    if mode == "compute" and i < NI - 1:
                        continue
                    nc.scalar.dma_start(
                        out=y[:, i * GB * T_LOC : (i + 1) * GB * T_LOC], in_=yt[:]
                    )
    nc.compile()
    return nc


DTYPE = "fp16"  # 'fp16' or 'f32r'

_NC_CACHE = None


def _get_nc():
    global _NC_CACHE
    if _NC_CACHE is None:
        _NC_CACHE = _build(dtype=DTYPE)
    return _NC_CACHE


def _prepare_in_maps(inputs, state, weight, bias):
    inputs = np.asarray(inputs, dtype=np.float32)
    state = np.asarray(state, dtype=np.float32)
    weight = np.asarray(weight, dtype=np.float32)
    bias = np.asarray(bias, dtype=np.float32)

    padded = np.concatenate([state, inputs], axis=0)  # [T+K, C]
    if DTYPE == "fp16":
        padded_r = padded.astype(np.float16)
        w_r = weight  # diag build rounds to fp16 on the DVE write
    else:
        padded_r = _round_fp32r(padded)
        w_r = _round_fp32r(weight)

    w_sb = np.ascontiguousarray(
        w_r.reshape(G, P, K).transpose(1, 0, 2).reshape(P, G * K)
    )
    b_sb = np.ascontiguousarray(bias.reshape(G, P).T)  # [P, G]
    ident = np.eye(P, dtype=np.float32)

    in_maps = []
    for m in range(N_CORES):
        seg = padded_r[m * T_LOC : m * T_LOC + TIN]  # [TIN, C]
        # x_flat[p, g*TIN + t] = seg[t, g*128 + p]
        xm = np.ascontiguousarray(
            seg.reshape(TIN, G, P).transpose(2, 1, 0).reshape(P, G * TIN)
        )
        in_maps.append({"x": xm, "w": w_sb, "b": b_sb, "ident": ident})
    return in_maps


def _assemble_out(results):
    """results[m]['y'] is [P, G*T_LOC] with y[p, g*T_LOC + t] = out[m*T_LOC+t, g*128+p]."""
    out = np.empty((T, C), dtype=np.float32)
    for m in range(N_CORES):
        yf = results[m]["y"].astype(np.float32).reshape(P, G, T_LOC)
        out[m * T_LOC : (m + 1) * T_LOC] = yf.transpose(2, 1, 0).reshape(T_LOC, C)
    return out


def kernel(inputs, state, weight, bias):
    nc = _get_nc()
    in_maps = _prepare_in_maps(inputs, state, weight, bias)
    res = run_bass_kernel_spmd(nc, in_maps, core_ids=list(range(N_CORES)))
    out = _assemble_out(res.results)

    split_points = np.cumsum(OUTPUT_DIMS[:-1]).tolist()
    split_outputs = tuple(
        np.ascontiguousarray(s) for s in np.split(out, split_points, axis=-1)
    )
    updated_state = np.ascontiguousarray(np.asarray(inputs, dtype=np.float32)[T - K : T])
    return (*split_outputs, updated_state)
